# revision 7
# baseline (speedup 1.0000x reference)
"""VQ codebook (vector-quantization) kernel for 8x Trainium2 NeuronCores.

Problem: z [32768, 512] f32, embedding [2048, 512] f32.
  d[n,k] = ||z_n||^2 + ||e_k||^2 - 2 z_n.e_k
  idx = argmin_k d  (fp32 reference semantics, first-index tie-break)
  z_q = e[idx]; z_q_st = z + (z_q - z);  loss = 1.25*mean((z_q-z)^2)
  perplexity from code-usage histogram.

Strategy:
  - Data-parallel: shard tokens 8 ways (4096/core), replicate codebook.
  - Device (per core): p = z @ (2e)^T via float32r matmuls (tf32-speed,
    ~fp32-ish precision), top-8 scores+indices per token via DVE max /
    max_index, indirect-DMA gather of e[top1], z_q_st and per-tile
    sum((z_q-z)^2) partials on DVE.
  - Host: tokens whose top1/top2 score gap is small are re-decided with
    an exact fp32 replication of the reference arithmetic (fp64 dots
    rounded to fp32, then the fp32 add/sub chain + first-index argmin).
    This reproduces the reference's fp32 grid-quantization ties exactly
    (validated: 0/32768 mismatches including 153 tie tokens).

Self-contained: hardcodes all shapes; no file reads.
"""

import os
import numpy as np

# ---- problem constants (hardcoded per contest rules) ----
N_TOKENS = 32768
DIM = 512
K_CODES = 2048
N_CORES = 8
SHARD = N_TOKENS // N_CORES          # 4096 tokens per core
P = 128                              # SBUF partitions
N_TILES = SHARD // P                 # 32 token tiles per core
D_CHUNKS = DIM // P                  # 4 contraction chunks
CODE_CHUNK = 512                     # matmul free dim (1 PSUM bank of f32)
N_CODE_CHUNKS = K_CODES // CODE_CHUNK  # 4
BETA = 0.25

# host refinement: flag tokens whose device (top1-top2) score gap is below
# this (score units: p = 2 z.e, spread ~0.013).  Covers fp32-grid tie
# ambiguity (~1.2e-4) + device tf32/bf16 noise (~1e-4) with >2x margin.
FLAG_GAP = 8e-4

_PROGRAM_CACHE = {}
LAST_EXEC_TIME_NS = None
LAST_RESULTS = None


def _build_program(shard=SHARD):
    """Build + compile the per-core Bass program (identical on all cores)."""
    from contextlib import ExitStack
    import concourse.bacc as bacc
    import concourse.tile as tile
    from concourse import mybir

    n_tiles = shard // P

    nc = bacc.Bacc(
        "TRN2",
        target_bir_lowering=False,
        debug=False,
        enable_asserts=False,
        num_devices=N_CORES,
    )
    f32 = mybir.dt.float32
    f32r = mybir.dt.float32r
    bf16 = mybir.dt.bfloat16
    u32 = mybir.dt.uint32

    # inputs (per core)
    zt_d = nc.dram_tensor("zt", [DIM, shard], f32r, kind="ExternalInput").ap()
    z_d = nc.dram_tensor("z", [shard, DIM], f32, kind="ExternalInput").ap()
    et2_d = nc.dram_tensor("et2", [DIM, K_CODES], f32r, kind="ExternalInput").ap()
    e_d = nc.dram_tensor("emb", [K_CODES, DIM], f32, kind="ExternalInput").ap()
    # outputs (per core)
    zqst_d = nc.dram_tensor("zqst", [shard, DIM], f32, kind="ExternalOutput").ap()
    v8_d = nc.dram_tensor("v8", [shard, 8], bf16, kind="ExternalOutput").ap()
    i8_d = nc.dram_tensor("i8", [shard, 8], u32, kind="ExternalOutput").ap()
    acc_d = nc.dram_tensor("acc", [P, n_tiles], f32, kind="ExternalOutput").ap()

    from concourse.bass import IndirectOffsetOnAxis

    with tile.TileContext(nc) as tc, ExitStack() as ctx:
        const = ctx.enter_context(tc.tile_pool(name="const", bufs=1))
        zpool = ctx.enter_context(tc.tile_pool(name="zp", bufs=3))
        score = ctx.enter_context(tc.tile_pool(name="sc", bufs=3))
        small = ctx.enter_context(tc.tile_pool(name="sm", bufs=4))
        dfp = ctx.enter_context(tc.tile_pool(name="df", bufs=3))
        psum = ctx.enter_context(tc.tile_pool(name="ps", bufs=2, space="PSUM"))

        # resident: zT (4x [128, shard]), eT2 (4x [128, K]), acc [128, n_tiles]
        zt_t = []
        et2_t = []
        for c in range(D_CHUNKS):
            zt = const.tile([P, shard], f32r, tag=f"zt{c}")
            nc.sync.dma_start(zt[:, :], zt_d[c * P:(c + 1) * P, :])
            zt_t.append(zt)
            et = const.tile([P, K_CODES], f32r, tag=f"et{c}")
            nc.sync.dma_start(et[:, :], et2_d[c * P:(c + 1) * P, :])
            et2_t.append(et)
        acc_t = const.tile([P, n_tiles], f32, tag="acc")

        for t in range(n_tiles):
            rows = slice(t * P, (t + 1) * P)
            ztile = zpool.tile([P, DIM], f32, tag="ztile")
            nc.sync.dma_start(ztile[:, :], z_d[rows, :])

            # scores p = z @ (2e)^T for this tile: one big PSUM tile (4 banks)
            pt = psum.tile([P, K_CODES], f32, tag="pmm")
            for ck in range(N_CODE_CHUNKS):
                cs = slice(ck * CODE_CHUNK, (ck + 1) * CODE_CHUNK)
                for dc in range(D_CHUNKS):
                    nc.tensor.matmul(
                        pt[:, cs],
                        lhsT=zt_t[dc][:, rows],
                        rhs=et2_t[dc][:, cs],
                        start=(dc == 0),
                        stop=(dc == D_CHUNKS - 1),
                    )
            # PSUM -> SBUF (+ cast to bf16): chunks 0-2 on ACT, chunk 3 on
            # DVE, so PSUM drains faster and PE stays dense
            sc_t = score.tile([P, K_CODES], bf16, tag="scores")
            nc.scalar.copy(sc_t[:, 0:3 * CODE_CHUNK], pt[:, 0:3 * CODE_CHUNK])
            nc.vector.tensor_copy(
                sc_t[:, 3 * CODE_CHUNK:], pt[:, 3 * CODE_CHUNK:]
            )

            v8 = small.tile([P, 8], bf16, tag="v8")
            i8 = small.tile([P, 8], u32, tag="i8")
            nc.vector.max(v8[:, :], sc_t[:, :])
            nc.vector.max_index(i8[:, :], v8[:, :], sc_t[:, :])

            # gather z_q = e[top1], then d = z_q - z on DVE
            zq = dfp.tile([P, DIM], f32, tag="zq")
            nc.gpsimd.indirect_dma_start(
                out=zq[:, :],
                out_offset=None,
                in_=e_d,
                in_offset=IndirectOffsetOnAxis(ap=i8[:, 0:1], axis=0),
            )
            d_t = dfp.tile([P, DIM], f32, tag="d")
            nc.gpsimd.tensor_sub(d_t[:, :], zq[:, :], ztile[:, :])

            # z_q_st = z + d on GPSIMD; loss partial = sum(d^2) on ACT
            o_t = dfp.tile([P, DIM], f32, tag="o")
            nc.gpsimd.tensor_add(o_t[:, :], ztile[:, :], d_t[:, :])
            sq = dfp.tile([P, DIM], f32, tag="sq")
            nc.scalar.activation(
                sq[:, :], d_t[:, :],
                mybir.ActivationFunctionType.Square,
                accum_out=acc_t[:, t:t + 1],
            )

            nc.sync.dma_start(zqst_d[rows, :], o_t[:, :])
            nc.sync.dma_start(v8_d[rows, :], v8[:, :])
            nc.sync.dma_start(i8_d[rows, :], i8[:, :])

        nc.sync.dma_start(acc_d[:, :], acc_t[:, :])

    nc.compile()
    return nc


def _get_program(shard=SHARD):
    if shard not in _PROGRAM_CACHE:
        _PROGRAM_CACHE[shard] = _build_program(shard)
    return _PROGRAM_CACHE[shard]


def _make_in_maps(z, e):
    et2 = np.ascontiguousarray((np.float32(2.0) * e).T)  # [512, 2048], exact 2x
    in_maps = []
    for c in range(N_CORES):
        zs = np.ascontiguousarray(z[c * SHARD:(c + 1) * SHARD])
        in_maps.append({
            "zt": np.ascontiguousarray(zs.T),
            "z": zs,
            "et2": et2,
            "emb": e,
        })
    return in_maps


def _run_device(z, e):
    global LAST_EXEC_TIME_NS, LAST_RESULTS
    from concourse.bass_utils import run_bass_kernel_spmd

    nc = _get_program()
    in_maps = _make_in_maps(z, e)
    trace = bool(int(os.environ.get("VQ_TRACE", "0")))
    res = run_bass_kernel_spmd(
        nc, in_maps, core_ids=list(range(N_CORES)), trace=trace
    )
    LAST_EXEC_TIME_NS = res.exec_time_ns
    LAST_RESULTS = res
    return res.results


def _host_combine(z, e, results):
    """Merge per-core outputs; exact-fp32 refinement of near-tie tokens."""
    zqst = np.concatenate([np.asarray(r["zqst"]) for r in results], axis=0)
    v8 = np.concatenate(
        [np.asarray(r["v8"]).astype(np.float32) for r in results], axis=0
    )
    i8 = np.concatenate(
        [np.asarray(r["i8"]).astype(np.int64) for r in results], axis=0
    )
    acc = np.stack([np.asarray(r["acc"]) for r in results], axis=0)

    idx = i8[:, 0].copy()
    loss_total = float(acc.astype(np.float64).sum())

    # ---- flag tokens that need exact re-decision ----
    gap = v8[:, 0] - v8[:, 1]
    bad = (i8 >= K_CODES).any(axis=1) | (idx < 0)
    flag = (gap < FLAG_GAP) | bad
    toks = np.nonzero(flag)[0]

    if toks.size:
        z64 = z[toks].astype(np.float64)
        cand = i8[toks].astype(np.int64)          # [F, 8]
        np.clip(cand, 0, K_CODES - 1, out=cand)
        e64 = e.astype(np.float64)
        ecand = e64[cand]                          # [F, 8, 512]
        s32 = np.einsum("fd,fkd->fk", z64, ecand).astype(np.float32)
        z2_32 = np.sum(z64 * z64, axis=1).astype(np.float32)   # [F]
        e2_32 = np.sum(e64 * e64, axis=1).astype(np.float32)   # [K]
        t32 = (z2_32[:, None] + e2_32[cand]).astype(np.float32)
        d32 = (t32 - (np.float32(2.0) * s32)).astype(np.float32)
        # mask out duplicate candidate slots (keep first occurrence)
        dup = np.zeros_like(d32, dtype=bool)
        for j in range(1, 8):
            dup[:, j] = (cand[:, j:j + 1] == cand[:, :j]).any(axis=1)
        d32[dup] = np.inf
        dmin = d32.min(axis=1)
        winner = np.where(d32 == dmin[:, None], cand, 2 ** 31).min(axis=1)

        changed = np.nonzero(winner != idx[toks])[0]
        if changed.size:
            ct = toks[changed]                    # token ids to fix
            new_i = winner[changed].astype(np.int64)
            old_i = idx[ct]
            # fix index
            idx[ct] = new_i
            # fix z_q_st rows with the exact fp32 chain
            zrow = z[ct]
            old_d = (e[old_i] - zrow).astype(np.float32)
            new_d = (e[new_i] - zrow).astype(np.float32)
            zqst[ct] = (zrow + new_d).astype(np.float32)
            # adjust loss partials
            loss_total += float(
                (new_d.astype(np.float64) ** 2).sum()
                - (old_d.astype(np.float64) ** 2).sum()
            )

    # ---- scalars ----
    m = np.float32(loss_total / (N_TOKENS * DIM))
    loss = np.float32(m + np.float32(np.float32(BETA) * m))

    counts = np.bincount(idx, minlength=K_CODES).astype(np.float64)
    avg_probs = counts / N_TOKENS
    perp = np.float32(np.exp(-np.sum(avg_probs * np.log(avg_probs + 1e-10))))

    return (
        zqst.astype(np.float32, copy=False),
        idx.astype(np.int32)[:, None],
        loss,
        perp,
    )


def kernel(**inputs):
    z = np.ascontiguousarray(np.asarray(inputs["z"], dtype=np.float32))
    e = np.ascontiguousarray(np.asarray(inputs["embedding"], dtype=np.float32))
    assert z.shape == (N_TOKENS, DIM) and e.shape == (K_CODES, DIM)
    results = _run_device(z, e)
    return _host_combine(z, e, results)


# revision 8
# speedup vs baseline: 1.2448x; 1.2448x over previous
"""VQ codebook (vector-quantization) kernel for 8x Trainium2 NeuronCores.

Problem: z [32768, 512] f32, embedding [2048, 512] f32.
  d[n,k] = ||z_n||^2 + ||e_k||^2 - 2 z_n.e_k
  idx = argmin_k d  (fp32 reference semantics, first-index tie-break)
  z_q = e[idx]; z_q_st = z + (z_q - z);  loss = 1.25*mean((z_q-z)^2)
  perplexity from code-usage histogram.

Strategy:
  - Data-parallel: shard tokens 8 ways (4096/core), replicate codebook.
  - Device (per core): p = z @ (2e)^T via float32r matmuls (tf32-speed,
    ~fp32-ish precision), top-8 scores+indices per token via DVE max /
    max_index, indirect-DMA gather of e[top1], z_q_st and per-tile
    sum((z_q-z)^2) partials on DVE.
  - Host: tokens whose top1/top2 score gap is small are re-decided with
    an exact fp32 replication of the reference arithmetic (fp64 dots
    rounded to fp32, then the fp32 add/sub chain + first-index argmin).
    This reproduces the reference's fp32 grid-quantization ties exactly
    (validated: 0/32768 mismatches including 153 tie tokens).

Self-contained: hardcodes all shapes; no file reads.
"""

import os
import numpy as np

# ---- problem constants (hardcoded per contest rules) ----
N_TOKENS = 32768
DIM = 512
K_CODES = 2048
N_CORES = 8
SHARD = N_TOKENS // N_CORES          # 4096 tokens per core
P = 128                              # SBUF partitions
N_TILES = SHARD // P                 # 32 token tiles per core
D_CHUNKS = DIM // P                  # 4 contraction chunks
CODE_CHUNK = 512                     # matmul free dim (1 PSUM bank of f32)
N_CODE_CHUNKS = K_CODES // CODE_CHUNK  # 4
BETA = 0.25

# host refinement: flag tokens whose device (top1-top2) score gap is below
# this (score units: p = 2 z.e, spread ~0.013).  Covers fp32-grid tie
# ambiguity (~1.2e-4) + device tf32/bf16 noise (~1e-4) with >2x margin.
FLAG_GAP = 8e-4

_PROGRAM_CACHE = {}
LAST_EXEC_TIME_NS = None
LAST_RESULTS = None


def _build_program(shard=SHARD):
    """Build + compile the per-core Bass program (identical on all cores)."""
    from contextlib import ExitStack
    import concourse.bacc as bacc
    import concourse.tile as tile
    from concourse import mybir

    n_tiles = shard // P

    nc = bacc.Bacc(
        "TRN2",
        target_bir_lowering=False,
        debug=False,
        enable_asserts=False,
        num_devices=N_CORES,
    )
    f32 = mybir.dt.float32
    f32r = mybir.dt.float32r
    bf16 = mybir.dt.bfloat16
    u32 = mybir.dt.uint32

    # inputs (per core)
    zt_d = nc.dram_tensor("zt", [DIM, shard], f32r, kind="ExternalInput").ap()
    z_d = nc.dram_tensor("z", [shard, DIM], f32, kind="ExternalInput").ap()
    et2_d = nc.dram_tensor("et2", [DIM, K_CODES], f32r, kind="ExternalInput").ap()
    e_d = nc.dram_tensor("emb", [K_CODES, DIM], f32, kind="ExternalInput").ap()
    # outputs (per core)
    zqst_d = nc.dram_tensor("zqst", [shard, DIM], f32, kind="ExternalOutput").ap()
    v8_d = nc.dram_tensor("v8", [shard, 8], bf16, kind="ExternalOutput").ap()
    i8_d = nc.dram_tensor("i8", [shard, 8], u32, kind="ExternalOutput").ap()
    acc_d = nc.dram_tensor("acc", [P, n_tiles], f32, kind="ExternalOutput").ap()

    from concourse.bass import IndirectOffsetOnAxis

    with tile.TileContext(nc) as tc, ExitStack() as ctx:
        const = ctx.enter_context(tc.tile_pool(name="const", bufs=1))
        zpool = ctx.enter_context(tc.tile_pool(name="zp", bufs=3))
        score = ctx.enter_context(tc.tile_pool(name="sc", bufs=3))
        small = ctx.enter_context(tc.tile_pool(name="sm", bufs=4))
        dfp = ctx.enter_context(tc.tile_pool(name="df", bufs=3))
        psum = ctx.enter_context(tc.tile_pool(name="ps", bufs=2, space="PSUM"))

        # resident: zT (4x [128, shard]), eT2 (4x [128, K]), acc [128, n_tiles]
        zt_t = []
        et2_t = []
        for c in range(D_CHUNKS):
            zt = const.tile([P, shard], f32r, tag=f"zt{c}")
            nc.sync.dma_start(zt[:, :], zt_d[c * P:(c + 1) * P, :])
            zt_t.append(zt)
            et = const.tile([P, K_CODES], f32r, tag=f"et{c}")
            nc.sync.dma_start(et[:, :], et2_d[c * P:(c + 1) * P, :])
            et2_t.append(et)
        acc_t = const.tile([P, n_tiles], f32, tag="acc")

        for t in range(n_tiles):
            rows = slice(t * P, (t + 1) * P)
            ztile = zpool.tile([P, DIM], f32, tag="ztile")
            nc.sync.dma_start(ztile[:, :], z_d[rows, :])

            # scores p = z @ (2e)^T for this tile: one big PSUM tile (4 banks)
            pt = psum.tile([P, K_CODES], f32, tag="pmm")
            for ck in range(N_CODE_CHUNKS):
                cs = slice(ck * CODE_CHUNK, (ck + 1) * CODE_CHUNK)
                for dc in range(D_CHUNKS):
                    nc.tensor.matmul(
                        pt[:, cs],
                        lhsT=zt_t[dc][:, rows],
                        rhs=et2_t[dc][:, cs],
                        start=(dc == 0),
                        stop=(dc == D_CHUNKS - 1),
                    )
            # single PSUM -> SBUF copy (+ cast to bf16) on the ACT engine
            sc_t = score.tile([P, K_CODES], bf16, tag="scores")
            nc.scalar.copy(sc_t[:, :], pt[:, :])

            v8 = small.tile([P, 8], bf16, tag="v8")
            i8 = small.tile([P, 8], u32, tag="i8")
            nc.vector.max(v8[:, :], sc_t[:, :])
            nc.vector.max_index(i8[:, :], v8[:, :], sc_t[:, :])

            # gather z_q = e[top1], then d = z_q - z on DVE
            zq = dfp.tile([P, DIM], f32, tag="zq")
            nc.gpsimd.indirect_dma_start(
                out=zq[:, :],
                out_offset=None,
                in_=e_d,
                in_offset=IndirectOffsetOnAxis(ap=i8[:, 0:1], axis=0),
            )
            d_t = dfp.tile([P, DIM], f32, tag="d")
            nc.gpsimd.tensor_sub(d_t[:, :], zq[:, :], ztile[:, :])

            # z_q_st = z + d on GPSIMD; loss partial = sum(d^2) on ACT
            o_t = dfp.tile([P, DIM], f32, tag="o")
            nc.gpsimd.tensor_add(o_t[:, :], ztile[:, :], d_t[:, :])
            sq = dfp.tile([P, DIM], f32, tag="sq")
            nc.scalar.activation(
                sq[:, :], d_t[:, :],
                mybir.ActivationFunctionType.Square,
                accum_out=acc_t[:, t:t + 1],
            )

            nc.sync.dma_start(zqst_d[rows, :], o_t[:, :])
            nc.sync.dma_start(v8_d[rows, :], v8[:, :])
            nc.sync.dma_start(i8_d[rows, :], i8[:, :])

        nc.sync.dma_start(acc_d[:, :], acc_t[:, :])

    nc.compile()
    return nc


def _get_program(shard=SHARD):
    if shard not in _PROGRAM_CACHE:
        _PROGRAM_CACHE[shard] = _build_program(shard)
    return _PROGRAM_CACHE[shard]


def _make_in_maps(z, e):
    et2 = np.ascontiguousarray((np.float32(2.0) * e).T)  # [512, 2048], exact 2x
    in_maps = []
    for c in range(N_CORES):
        zs = np.ascontiguousarray(z[c * SHARD:(c + 1) * SHARD])
        in_maps.append({
            "zt": np.ascontiguousarray(zs.T),
            "z": zs,
            "et2": et2,
            "emb": e,
        })
    return in_maps


def _run_device(z, e):
    global LAST_EXEC_TIME_NS, LAST_RESULTS
    from concourse.bass_utils import run_bass_kernel_spmd

    nc = _get_program()
    in_maps = _make_in_maps(z, e)
    trace = bool(int(os.environ.get("VQ_TRACE", "0")))
    res = run_bass_kernel_spmd(
        nc, in_maps, core_ids=list(range(N_CORES)), trace=trace
    )
    LAST_EXEC_TIME_NS = res.exec_time_ns
    LAST_RESULTS = res
    return res.results


def _host_combine(z, e, results):
    """Merge per-core outputs; exact-fp32 refinement of near-tie tokens."""
    zqst = np.concatenate([np.asarray(r["zqst"]) for r in results], axis=0)
    v8 = np.concatenate(
        [np.asarray(r["v8"]).astype(np.float32) for r in results], axis=0
    )
    i8 = np.concatenate(
        [np.asarray(r["i8"]).astype(np.int64) for r in results], axis=0
    )
    acc = np.stack([np.asarray(r["acc"]) for r in results], axis=0)

    idx = i8[:, 0].copy()
    loss_total = float(acc.astype(np.float64).sum())

    # ---- flag tokens that need exact re-decision ----
    gap = v8[:, 0] - v8[:, 1]
    bad = (i8 >= K_CODES).any(axis=1) | (idx < 0)
    flag = (gap < FLAG_GAP) | bad
    toks = np.nonzero(flag)[0]

    if toks.size:
        z64 = z[toks].astype(np.float64)
        cand = i8[toks].astype(np.int64)          # [F, 8]
        np.clip(cand, 0, K_CODES - 1, out=cand)
        e64 = e.astype(np.float64)
        ecand = e64[cand]                          # [F, 8, 512]
        s32 = np.einsum("fd,fkd->fk", z64, ecand).astype(np.float32)
        z2_32 = np.sum(z64 * z64, axis=1).astype(np.float32)   # [F]
        e2_32 = np.sum(e64 * e64, axis=1).astype(np.float32)   # [K]
        t32 = (z2_32[:, None] + e2_32[cand]).astype(np.float32)
        d32 = (t32 - (np.float32(2.0) * s32)).astype(np.float32)
        # mask out duplicate candidate slots (keep first occurrence)
        dup = np.zeros_like(d32, dtype=bool)
        for j in range(1, 8):
            dup[:, j] = (cand[:, j:j + 1] == cand[:, :j]).any(axis=1)
        d32[dup] = np.inf
        dmin = d32.min(axis=1)
        winner = np.where(d32 == dmin[:, None], cand, 2 ** 31).min(axis=1)

        changed = np.nonzero(winner != idx[toks])[0]
        if changed.size:
            ct = toks[changed]                    # token ids to fix
            new_i = winner[changed].astype(np.int64)
            old_i = idx[ct]
            # fix index
            idx[ct] = new_i
            # fix z_q_st rows with the exact fp32 chain
            zrow = z[ct]
            old_d = (e[old_i] - zrow).astype(np.float32)
            new_d = (e[new_i] - zrow).astype(np.float32)
            zqst[ct] = (zrow + new_d).astype(np.float32)
            # adjust loss partials
            loss_total += float(
                (new_d.astype(np.float64) ** 2).sum()
                - (old_d.astype(np.float64) ** 2).sum()
            )

    # ---- scalars ----
    m = np.float32(loss_total / (N_TOKENS * DIM))
    loss = np.float32(m + np.float32(np.float32(BETA) * m))

    counts = np.bincount(idx, minlength=K_CODES).astype(np.float64)
    avg_probs = counts / N_TOKENS
    perp = np.float32(np.exp(-np.sum(avg_probs * np.log(avg_probs + 1e-10))))

    return (
        zqst.astype(np.float32, copy=False),
        idx.astype(np.int32)[:, None],
        loss,
        perp,
    )


def kernel(**inputs):
    z = np.ascontiguousarray(np.asarray(inputs["z"], dtype=np.float32))
    e = np.ascontiguousarray(np.asarray(inputs["embedding"], dtype=np.float32))
    assert z.shape == (N_TOKENS, DIM) and e.shape == (K_CODES, DIM)
    results = _run_device(z, e)
    return _host_combine(z, e, results)
